# revision 74
# baseline (speedup 1.0000x reference)
"""Trainium2 Bass kernel for causal multi-head self-attention.

Problem: nn_MultiHeadSelfAttention (B=2, T=2048, D=768, H=12, HD=64).

    qkv = x @ Wqkv ; per-head causal softmax(q k^T / sqrt(hd)) @ v ; out @ Wo + bo

Sharding (8 cores): data-parallel over B (2) x tensor-parallel over heads
(4 groups of 3 heads).  Each core computes the QKV projection for its own
head slice, runs attention for its 3 heads, and produces a partial o_proj
output [T, D] (rows of Wo for its heads).  Host sums the 4 partials per
batch and adds the bias.

Design notes (~140us vs the 205us v1 baseline; key measured facts):
  - The PE is power/HAM-clamped to 1.2 GHz from ~95-110us into every run
    regardless of structure, so the schedule packs the PE-densest work
    early and minimizes post-clamp PE cycles.
  - Input DMA rides three queues (sync/scalar/gpsimd), each carrying its
    k-tiles' weights first, so projection starts ~10us in (7us is fixed
    engine boot) and DMA hides under compute.
  - Phase A projects Q0/Q1/K0/K1 for token chunks 0-1 only; the rest of
    the projection (chunks 2-3, mt2, V) is PE filler inside the attention
    slot loops so the PE stream never drains (draining re-throttles HAM).
  - Pair phase: heads 0+1 per key-tile with score matmuls on disjoint PE
    row groups (rows 0-63 / 64-127) running concurrently, in query-half
    sweeps so PSUM fits: a 4-deep ring of 1-bank score strips + 4 window
    accumulators.  The 4-deep ring + 3-slot score lookahead (pair) and
    2-slot-pair prefetch (head 2) are the big pipelining levers; exp is
    one ACT instruction per strip and ACT does nothing but exp.
  - The causal mask is applied AFTER exp as a 0/1 bf16 multiply on P^T
    (exp of unmasked scores is finite), keeping the DVE off the
    PE->ACT critical path.
  - Head-2 phase: odd key-tiles run on rows 64-127 via staged Qh2/Kh2
    copies, pairing with even tiles; o_proj rides as filler (heads 0+1
    partials were already emitted during the pair phase via emit_po01).
  - Softmax denominators ride PV as a ones-column (PSUM row 64);
    normalization = reciprocal_approx_fast (SBUF-staged: the custom-DVE
    op reads garbage from PSUM) + gpsimd partition_broadcast + one DVE
    multiply fused into the PSUM->SBUF O^T write.
  - fp8 operands were tried and measured 6% rms error (dot-product
    rounding noise does not average down with contraction length): bf16
    everywhere.
"""

import os
import sys

for _p in ("/opt/trn_rl_repo",):
    if os.path.isdir(_p) and _p not in sys.path:
        sys.path.insert(0, _p)

import numpy as np
import ml_dtypes

import concourse.bass as bass
import concourse.mybir as mybir
import concourse.tile as tile
from concourse import bacc
from concourse.bass_utils import run_bass_kernel_spmd
from concourse.masks import make_identity, make_lower_triangular

F32 = mybir.dt.float32

MM_DT = mybir.dt.bfloat16
NP_IN = ml_dtypes.bfloat16

B, T, D, H = 2, 2048, 768, 12
HD = 64
HPC = 3            # heads per core
GROUPS = 4         # head groups (tensor-parallel)
N_CORES = 8
KT = D // 128      # 6 k-tiles over the feature dim
QKCOLS = HPC * 2 * HD  # 384 projected q/k columns
VC = HPC * HD          # 192 v columns
SCALE = 1.0 / np.sqrt(HD)
NCHUNK = 512
NQW = T // NCHUNK  # 4 query windows
NKJ = T // 128     # 16 key tiles
VBW = HD + 2       # v block width incl. ones columns
SW = 512           # exp strip width (1 PSUM bank; 4-deep ring)
PTW = 1024         # pair-phase PT width (query-half)
NEG = -1.0e30

_CACHE = {}


def _build_program():
    """Build the per-core Bass program (identical on all cores)."""
    nc = bacc.Bacc("TRN2", target_bir_lowering=False, debug=False,
                   num_devices=N_CORES, name="mhsa")

    # fp8 operands measure ~6% rms on this data (dot-product noise does not
    # average down); bf16 it is.
    xT_d = nc.dram_tensor("xT", [128, KT * T], MM_DT, kind="ExternalInput").ap()
    wqk_d = nc.dram_tensor("wqk", [128, KT * QKCOLS], MM_DT,
                           kind="ExternalInput").ap()
    wv_d = nc.dram_tensor("wv", [128, KT * VC], MM_DT,
                          kind="ExternalInput").ap()
    wo_d = nc.dram_tensor("wo", [VC, D], MM_DT, kind="ExternalInput").ap()
    out_d = nc.dram_tensor("out", [T, D], F32, kind="ExternalOutput").ap()

    with tile.TileContext(nc) as tc:
        with (
            tc.tile_pool(name="const", bufs=1) as const,
            tc.tile_pool(name="persist", bufs=1) as persist,
            tc.tile_pool(name="work", bufs=6) as work,
            tc.tile_pool(name="obp", bufs=4) as obp,
            tc.tile_pool(name="ptp", bufs=8) as ptp,
            tc.tile_pool(name="ptp2", bufs=6) as ptp2,
            tc.tile_pool(name="strips", bufs=4, space="PSUM") as strips,
            tc.tile_pool(name="psacc", bufs=4, space="PSUM") as psacc,
        ):
            # ---- persistent inputs, DMA'd across three queues, each
            # carrying its k-tiles' qk-weight slice first so projection for
            # token-chunk 0 can start as early as possible.
            QUEUES = [nc.sync, nc.scalar, nc.gpsimd]
            wqk_s = persist.tile([128, KT * QKCOLS], MM_DT, tag="wqk")
            wqk8 = wqk_s.rearrange("p (k m) -> p k m", m=QKCOLS)
            xT_s = persist.tile([128, KT * T], MM_DT, tag="xT")
            xT8 = xT_s.rearrange("p (k t) -> p k t", t=T)
            for k in range(KT):
                QUEUES[k % 3].dma_start(
                    wqk8[:, k, :],
                    wqk_d[:, k * QKCOLS:(k + 1) * QKCOLS])
            wv_s = persist.tile([128, KT * VC], MM_DT, tag="wv")
            wv8 = wv_s.rearrange("p (k m) -> p k m", m=VC)
            for n0 in range(0, T, NCHUNK):
                for k in range(KT):
                    QUEUES[k % 3].dma_start(
                        xT8[:, k, n0:n0 + NCHUNK],
                        xT_d[:, k * T + n0:k * T + n0 + NCHUNK])
                if n0 == NCHUNK:
                    # v weights right after token chunks 0-1: the V
                    # projection starts as pair-phase filler ~6us in
                    for k in range(KT):
                        QUEUES[k % 3].dma_start(wv8[:, k, :],
                                                wv_d[:, k * VC:(k + 1) * VC])
            wo01 = persist.tile([128, D], MM_DT, tag="wo01")
            nc.gpsimd.dma_start(wo01, wo_d[0:128, :])
            wo2 = persist.tile([64, D], MM_DT, tag="wo2")
            nc.gpsimd.dma_start(wo2, wo_d[128:VC, :])
            # ---- constants ----
            maskneg = const.tile([128, 128], F32, tag="maskneg")
            make_lower_triangular(nc, maskneg, val=NEG, diag=False)
            ones_f = const.tile([128, 2], F32, tag="ones_f")
            nc.gpsimd.memset(ones_f, 1.0)
            ones64 = const.tile([1, 64], F32, tag="ones64")
            nc.gpsimd.memset(ones64, 1.0)
            ones_t = const.tile([128, 2], MM_DT, tag="ones_t")
            nc.vector.tensor_copy(ones_t, ones_f)
            ident_f = const.tile([128, 128], F32, tag="ident_f")
            make_identity(nc, ident_f)

            # preload the exp table set while input DMA runs (after the
            # scalar queue's DMA emissions so the table load doesn't block
            # them)
            expwarm = const.tile([1, 2], F32, tag="expwarm")
            nc.scalar.activation(expwarm, ones_f[0:1, 0:2],
                                 mybir.ActivationFunctionType.Exp, scale=1.0)

            # ---- persistent intermediates ----
            mt = [persist.tile([128, T], MM_DT, tag=f"mt{m}", name=f"mt{m}")
                  for m in range(3)]
            kt2 = persist.tile([64, T], MM_DT, tag="kt2")   # Kh2 re-based to 0
            qkt2 = persist.tile([128, T], MM_DT, tag="qkt2")  # Qh2 on 64-127
            V_t = []
            for h in range(HPC):
                vt = persist.tile([128, NKJ * VBW], MM_DT, tag=f"V{h}")
                vt3 = vt.rearrange("p (j c) -> p j c", c=VBW)
                nc.vector.tensor_copy(
                    vt3[:, :, HD:HD + 2],
                    ones_t.unsqueeze(1).to_broadcast((128, NKJ, 2)))
                V_t.append(vt)
            # normalized O^T: heads 0,1 stacked on partitions; head 2 alone
            OT01 = persist.tile([128, T], MM_DT, tag="OT01")
            OT2 = persist.tile([64, T], MM_DT, tag="OT2")

            # ---- phase A: project Q0/Q1 and K0/K1 only (token-chunk-outer
            # so DMA pipelines).  mt2 and the V projection are deferred into
            # the attention slot loop as PE filler work.
            def emit_mtchunk(m, n0):
                ps = strips.tile([128, SW], F32, tag="strip", name="psqk")
                for k in range(KT):
                    nc.tensor.matmul(
                        ps[:, :NCHUNK],
                        lhsT=wqk8[:, k, m * 128:(m + 1) * 128],
                        rhs=xT8[:, k, n0:n0 + NCHUNK],
                        start=(k == 0), stop=(k == KT - 1),
                    )
                nc.vector.tensor_copy(mt[m][:, n0:n0 + NCHUNK],
                                      ps[:, :NCHUNK])

            def emit_v(j):
                pv = strips.tile([128, SW], F32, tag="strip", name="pv")
                for k in range(KT):
                    nc.tensor.matmul(
                        pv[:, :VC],
                        lhsT=xT8[:, k, j * 128:(j + 1) * 128],
                        rhs=wv8[:, k, :],
                        start=(k == 0), stop=(k == KT - 1),
                    )
                for h in range(HPC):
                    nc.vector.tensor_copy(
                        V_t[h][:, j * VBW:j * VBW + HD],
                        pv[:, h * HD:(h + 1) * HD])

            # only token chunks 0-1: query-half 0 of the pair phase needs
            # no more; chunks 2-3 are emitted as pair-phase filler
            for n0 in (0, NCHUNK):
                for m in range(2):
                    emit_mtchunk(m, n0)

            # head views: (Q, K)
            heads = [
                (mt[0][0:64, :], mt[1][0:64, :]),
                (mt[0][64:128, :], mt[1][64:128, :]),
                (mt[2][0:64, :], kt2[0:64, :]),
            ]

            # ---- attention ----
            def emit_fin(h, qw, acc):
                """Normalize O^T for (h, window) out of PSUM into SBUF."""
                den = work.tile([1, NCHUNK], F32, tag="den", name="den")
                nc.vector.tensor_copy(den, acc[64:65, :])
                rr = work.tile([1, NCHUNK], F32, tag="rr", name="rr")
                nc.vector.reciprocal_approx_fast(rr, den)
                rrb = work.tile([64, NCHUNK], F32, tag="rrb", name="rrb")
                nc.gpsimd.partition_broadcast(rrb, rr)
                if h == 0:
                    dst = OT01[0:64, qw * NCHUNK:(qw + 1) * NCHUNK]
                elif h == 1:
                    dst = OT01[64:128, qw * NCHUNK:(qw + 1) * NCHUNK]
                else:
                    dst = OT2[0:64, qw * NCHUNK:(qw + 1) * NCHUNK]
                nc.vector.tensor_tensor(dst, acc[0:64, :], rrb,
                                        mybir.AluOpType.mult)

            oba = {}

            def emit_po01(tt):
                """Partial o_proj (heads 0+1 only) for one token tile,
                stashed in SBUF f32; head 2 is added later in its phase."""
                oba[tt] = persist.tile([128, D], F32, tag=f"oba{tt}",
                                       name=f"oba{tt}")
                for n0, nw in ((0, 512), (512, 256)):
                    po = strips.tile([128, SW], F32, tag="strip", name="po")
                    nc.tensor.matmul(
                        po[:, :nw],
                        lhsT=OT01[:, tt * 128:(tt + 1) * 128],
                        rhs=wo01[:, n0:n0 + nw],
                        start=True, stop=True,
                    )
                    nc.vector.tensor_copy(oba[tt][:, n0:n0 + nw],
                                          po[:, :nw])

            def emit_po2(tt):
                """Head-2 o_proj for one token tile + combine with the
                stashed heads-0+1 partial, then DMA out."""
                ob = obp.tile([128, D], F32, tag="ob", name="ob")
                for n0, nw in ((0, 512), (512, 256)):
                    po = strips.tile([128, SW], F32, tag="strip", name="po")
                    nc.tensor.matmul(
                        po[:, :nw],
                        lhsT=OT2[:, tt * 128:(tt + 1) * 128],
                        rhs=wo2[:, n0:n0 + nw],
                        start=True, stop=True,
                    )
                    nc.vector.tensor_tensor(ob[:, n0:n0 + nw], po[:, :nw],
                                            oba[tt][:, n0:n0 + nw],
                                            mybir.AluOpType.add)
                QUEUES[tt % 3].dma_start(out_d[tt * 128:(tt + 1) * 128, :],
                                         ob)

            rT = work.tile([128, 4], F32, tag="rT", name="rT", bufs=1)

            def emit_fin_last(acc):
                """h2 final window: write O^T unnormalized and PE-transpose
                the reciprocal row to token-partitions; normalization rides
                the o_proj combine as a per-partition scalar.  The PE is
                idle here and the score-strip ring is drained."""
                den = work.tile([1, NCHUNK], F32, tag="den", name="den")
                nc.vector.tensor_copy(den, acc[64:65, :])
                rr = work.tile([1, NCHUNK], F32, tag="rr", name="rr")
                nc.vector.reciprocal_approx_fast(rr, den)
                nc.vector.tensor_copy(
                    OT2[0:64, 3 * NCHUNK:4 * NCHUNK], acc[0:64, :])
                pden = strips.tile([128, SW], F32, tag="strip", name="pden")
                for j in range(4):
                    nc.tensor.transpose(pden[:, j:j + 1],
                                        rr[0:1, j * 128:(j + 1) * 128],
                                        ident_f[0:1, 0:1])
                nc.vector.tensor_copy(rT, pden[:, 0:4])

            def emit_po2_last(tt):
                ob = obp.tile([128, D], F32, tag="ob", name="ob")
                for n0, nw in ((0, 512), (512, 256)):
                    po = strips.tile([128, SW], F32, tag="strip", name="po")
                    nc.tensor.matmul(
                        po[:, :nw],
                        lhsT=OT2[:, tt * 128:(tt + 1) * 128],
                        rhs=wo2[:, n0:n0 + nw],
                        start=True, stop=True,
                    )
                    nc.vector.scalar_tensor_tensor(
                        ob[:, n0:n0 + nw], po[:, :nw],
                        rT[:, tt - 12:tt - 11], oba[tt][:, n0:n0 + nw],
                        op0=mybir.AluOpType.mult, op1=mybir.AluOpType.add)
                QUEUES[tt % 3].dma_start(out_d[tt * 128:(tt + 1) * 128, :],
                                         ob)

            deferred = []

            def flush_deferred(limit=None):
                n = 0
                while deferred and (limit is None or n < limit):
                    deferred.pop(0)()
                    n += 1

            def emit_scores_q(h, kj, qb, qe):
                """Scores for (head, key-tile) restricted to q in [qb, qe),
                strip units of <=SW columns, one exp each; returns PT (col
                0 <-> q = qb)."""
                Qh, Kh = heads[h]
                q0 = 128 * kj
                PT = ptp.tile([128, PTW], MM_DT, tag="pt", name="pt")
                qa = qb
                while qa < qe:
                    uw = min(SW, qe - qa)
                    strip = strips.tile([128, SW], F32, tag="strip",
                                        name="sc")
                    qp = qa
                    while qp < qa + uw:
                        pw = min(NCHUNK, qa + uw - qp)
                        nc.tensor.matmul(
                            strip[:, qp - qa:qp - qa + pw],
                            lhsT=Kh[:, q0:q0 + 128],
                            rhs=Qh[:, qp:qp + pw],
                            start=True, stop=True,
                        )
                        qp += pw
                    if qa == q0:
                        # causal mask onto the diagonal block, in PSUM
                        nc.vector.tensor_add(strip[:, 0:128],
                                             strip[:, 0:128], maskneg)
                    nc.scalar.activation(
                        PT[:, qa - qb:qa - qb + uw], strip[:, :uw],
                        mybir.ActivationFunctionType.Exp,
                        scale=float(SCALE))
                    qa += uw
                return PT

            def emit_pv_q(h, kj, qb, qw, acc, PT, start):
                """PV accumulation for global window qw out of a PT whose
                col 0 is q = qb."""
                qs = max(qw * NCHUNK, qb)
                lo = qs - qw * NCHUNK
                nc.tensor.matmul(
                    acc[:66, lo:NCHUNK],
                    lhsT=V_t[h][:, kj * VBW:(kj + 1) * VBW],
                    rhs=PT[:, qs - qb:qs - qb + NCHUNK - lo],
                    start=start, stop=(kj == 4 * qw + 3),
                )

            # ---- pair phase: heads 0+1 together, query-half sweeps ----
            # Per (qhalf, key-tile) slot both heads' score matmuls are
            # emitted back-to-back: they target disjoint PE row groups
            # (rows 0-63 vs 64-127) so the array runs them concurrently.
            # Fillers (V projection, mt2, partial o_proj) pad the PE.
            acc_tiles = {}
            pair_slots = [(qh, kj) for qh in range(2)
                          for kj in range((qh + 1) * 8)]

            def pair_scores(qh, kj):
                qhs = qh * 2 * NCHUNK
                qb = max(128 * kj, qhs)
                qe = qhs + 2 * NCHUNK
                return (emit_scores_q(0, kj, qb, qe),
                        emit_scores_q(1, kj, qb, qe))

            def h2_scores(kj):
                """One PT spanning q in [128*kj, T); strip units of <=SW.
                Odd key-tiles run on PE rows 64-127 (staged Q/K copies) so
                they execute concurrently with even ones on rows 0-63."""
                if kj % 2 == 0:
                    Qh, Kh = heads[2]
                else:
                    Qh, Kh = qkt2[64:128, :], mt[2][64:128, :]
                q0 = 128 * kj
                PT = ptp2.tile([128, T], MM_DT, tag="pt2", name="pt2")
                qa = q0
                while qa < T:
                    uw = min(SW, T - qa)
                    strip = strips.tile([128, SW], F32, tag="strip",
                                        name="sc")
                    qp = qa
                    while qp < qa + uw:
                        pw = min(NCHUNK, qa + uw - qp)
                        nc.tensor.matmul(
                            strip[:, qp - qa:qp - qa + pw],
                            lhsT=Kh[:, q0:q0 + 128],
                            rhs=Qh[:, qp:qp + pw],
                            start=True, stop=True,
                        )
                        qp += pw
                    if qa == q0:
                        nc.vector.tensor_add(strip[:, 0:128],
                                             strip[:, 0:128], maskneg)
                    nc.scalar.activation(
                        PT[:, qa - q0:qa - q0 + uw], strip[:, :uw],
                        mybir.ActivationFunctionType.Exp,
                        scale=float(SCALE))
                    qa += uw
                return PT

            h2_pt = {}
            for j in range(5):
                emit_v(j)
            pair_pt = {}
            for s in pair_slots[:3]:
                pair_pt[s] = pair_scores(*s)
            for i, (qh, kj) in enumerate(pair_slots):
                # fillers
                if qh == 0 and kj < 3:
                    emit_v(kj + 5)
                elif qh == 0 and 3 <= kj <= 6:
                    emit_mtchunk((kj - 3) % 2, (2 + (kj - 3) // 2) * NCHUNK)
                elif qh == 0 and kj == 7:
                    emit_mtchunk(2, 0)
                elif qh == 1 and kj < 3:
                    emit_mtchunk(2, (kj + 1) * NCHUNK)
                elif qh == 1 and kj == 3:
                    # re-base Kh2 (partitions 64-127 of mt2) to partition 0
                    # and stage a copy of Qh2 on partitions 64-127 so head
                    # 2's odd key-tiles can run on the upper PE row groups
                    nc.sync.dma_start(kt2, mt[2][64:128, :])
                    nc.sync.dma_start(qkt2[64:128, :], mt[2][0:64, :])
                elif qh == 1 and 4 <= kj <= 11:
                    emit_v(kj + 4)
                elif qh == 1 and kj >= 12:
                    # prime head 2's first score tiles (one per slot: each
                    # is several strip units at SW=512) to bridge the phase
                    # boundary without draining the PE queue
                    h2_pt[kj - 12] = h2_scores(kj - 12)
                if i + 3 < len(pair_slots):
                    s = pair_slots[i + 3]
                    pair_pt[s] = pair_scores(*s)
                flush_deferred(limit=2)
                PTs = pair_pt.pop((qh, kj))
                qhs = qh * 2 * NCHUNK
                qb = max(128 * kj, qhs)
                for h in (0, 1):
                    for qw in (2 * qh, 2 * qh + 1):
                        if 4 * qw + 3 < kj:
                            continue
                        if kj == 0:
                            acc_tiles[(h, qw)] = psacc.tile(
                                [128, NCHUNK], F32, tag="acc",
                                name=f"acc{h}{qw}")
                        emit_pv_q(h, kj, qb, qw, acc_tiles[(h, qw)],
                                  PTs[h], start=(kj == 0))
                if kj % 4 == 3:
                    qw = kj // 4
                    if 2 * qh <= qw <= 2 * qh + 1:
                        for h in (0, 1):
                            acc = acc_tiles.pop((h, qw))
                            deferred.append(lambda h=h, qw=qw, acc=acc:
                                            emit_fin(h, qw, acc))
                        for tt in range(4 * qw, 4 * qw + 4):
                            deferred.append(lambda tt=tt: emit_po01(tt))
            # ---- head 2 phase: key-tile pairs on disjoint PE row groups
            # (even kj on rows 0-63 via kt2, odd kj on rows 64-127 via the
            # staged Qh2/Kh2 copies) so scores run two-at-a-time; o_proj
            # rides as filler.
            for kj2 in range(NKJ // 2):
                for kj in (2 * kj2 + 2, 2 * kj2 + 3,
                           2 * kj2 + 4, 2 * kj2 + 5):
                    if kj < NKJ and kj not in h2_pt:
                        h2_pt[kj] = h2_scores(kj)
                flush_deferred(limit=3)
                for kj in (2 * kj2, 2 * kj2 + 1):
                    q0 = 128 * kj
                    PT = h2_pt.pop(kj)
                    for qw in range(kj // 4, NQW):
                        if kj == 0:
                            acc_tiles[(2, qw)] = psacc.tile(
                                [128, NCHUNK], F32, tag="acc",
                                name=f"acc2{qw}")
                        emit_pv_q(2, kj, q0, qw, acc_tiles[(2, qw)], PT,
                                  start=(kj == 0))
                    if kj % 4 == 3:
                        qw = kj // 4
                        acc = acc_tiles.pop((2, qw))
                        if qw == 3:
                            deferred.append(lambda acc=acc:
                                            emit_fin_last(acc))
                            for tt in range(12, 16):
                                deferred.append(
                                    lambda tt=tt: emit_po2_last(tt))
                        else:
                            deferred.append(lambda qw=qw, acc=acc:
                                            emit_fin(2, qw, acc))
                            for tt in range(qw * 4, qw * 4 + 4):
                                deferred.append(lambda tt=tt: emit_po2(tt))
            flush_deferred()

    nc.compile()
    return nc


def _get_program():
    if "nc" not in _CACHE:
        _CACHE["nc"] = _build_program()
    return _CACHE["nc"]


def _shard_inputs(x, Wqkv, Wo):
    """Build the 8 per-core input maps."""
    in_maps = []
    for c in range(N_CORES):
        b, hg = divmod(c, GROUPS)
        h0 = HPC * hg
        def qcol(h):
            return Wqkv[:, (h0 + h) * HD:(h0 + h + 1) * HD]
        def kcol(h):
            return Wqkv[:, D + (h0 + h) * HD:D + (h0 + h + 1) * HD]
        def vcol(h):
            return Wqkv[:, 2 * D + (h0 + h) * HD:2 * D + (h0 + h + 1) * HD]
        # mt0=[Qh0|Qh1] mt1=[Kh0|Kh1] mt2=[Qh2|Kh2]
        wqk = np.concatenate([qcol(0), qcol(1), kcol(0), kcol(1),
                              qcol(2), kcol(2)], axis=1)
        wv = np.concatenate([vcol(0), vcol(1), vcol(2)], axis=1)

        def pack8(a):
            """[D, M] -> [128, KT*M] fp8, k-tiles side by side."""
            m = a.shape[1]
            return np.ascontiguousarray(
                a.reshape(KT, 128, m).transpose(1, 0, 2).reshape(128, KT * m)
            ).astype(NP_IN)

        in_maps.append({
            "xT": pack8(np.ascontiguousarray(x[b].T)),
            "wqk": pack8(wqk),
            "wv": pack8(wv),
            "wo": np.ascontiguousarray(
                Wo[h0 * HD:(h0 + HPC) * HD, :]).astype(NP_IN),
        })
    return in_maps


def kernel(x, attn_mask, Wqkv, Wo, bo):
    x = np.asarray(x, dtype=np.float32)
    Wqkv = np.asarray(Wqkv, dtype=np.float32)
    Wo = np.asarray(Wo, dtype=np.float32)
    bo = np.asarray(bo, dtype=np.float32)
    # attn_mask is causal by construction; causality is hardcoded on-device.

    nc = _get_program()
    in_maps = _shard_inputs(x, Wqkv, Wo)

    res = run_bass_kernel_spmd(nc, in_maps, core_ids=list(range(N_CORES)),
                               **_CACHE.get("run_kwargs", {}))
    _CACHE["last_results"] = res

    out = np.zeros((B, T, D), dtype=np.float32)
    for c in range(N_CORES):
        b = c // GROUPS
        out[b] += res.results[c]["out"]
    out += bo[None, None, :]
    return out
